# revision 4
# baseline (speedup 1.0000x reference)
"""Trainium2 Bass kernel for nn_LoRALinear (quantized linear + LoRA).

reference:
    w_dq = quant_dequant_int8_per_row(weight)          # [out, in]
    out  = x @ w_dq.T + (alpha/r) * (x @ la) @ lb      # [T, out]

Math identity used here:
    out = x @ (w_dq.T + 2.0 * (la @ lb)) = x @ W_eff
        = diag(s_tok) . (x_q @ W_eff)        with x ~= diag(s_tok) . x_q

Host prep folds quant-dequant + LoRA into W_eff, re-quantizes W_eff to int8
with per-out-feature scales, and quantizes x to int8 with per-token scales
(which factor exactly through the matmul and are re-applied on the PSUM
evacuation). This keeps host->device traffic at 24 MiB/core in + 16 MiB/core
out (vs 48+32 for a bf16-everything kernel) while the device math runs in
bf16 with fp32 PSUM accumulation.

Per-core device program (SPMD identical, data-parallel over tokens):
    xt  [4, 128, 32, 512] int8  - token-shard quarters (SBUF layout)
    xs  [128, 16]         f32   - per-token scales ([p, m-tile])
    wq  [8, 128, 32, 512] int8  - W_eff stripes, replicated
    ws  [8, 512]          bf16  - per-out-feature scales (DMA-broadcast)
    out [2048, 4096]      bf16  - host upcasts to f32
Stripes are dequantized to bf16 on the vector engine; x quarters are
converted int8->bf16 (exact) on the vector engine; PSUM tiles are scaled by
s_tok and cast to bf16 on the scalar engine. End-to-end rel err vs the f32
reference is ~1.1e-2 (budget 2e-2).
"""

import numpy as np
import ml_dtypes

TOKENS, IN_F, OUT_F, R = 16384, 4096, 4096, 16
N_CORES = 8
TPC = TOKENS // N_CORES  # tokens per core: 2048
SCALING = 2.0  # alpha / r
P = 128
NS = 512  # out_feature stripe (one PSUM bank of f32)
QN = 4    # x quarters
QT = TPC // QN  # 512 tokens per quarter

_NC_CACHE = {}


def _build_nc(repeat=1):
    import concourse.mybir as mybir
    import concourse.tile as tile
    from concourse import bacc

    nc = bacc.Bacc("TRN2", target_bir_lowering=False)
    ko_n = IN_F // P    # 32
    ns = NS
    nt_n = OUT_F // ns  # 8
    mo_n = QT // P      # 4
    mt_n = TPC // P     # 16

    xt = nc.dram_tensor("xt", [QN, P, ko_n, QT], mybir.dt.int8, kind="ExternalInput")
    xs = nc.dram_tensor("xs", [P, mt_n], mybir.dt.float32, kind="ExternalInput")
    wq = nc.dram_tensor("wq", [nt_n, P, ko_n, ns], mybir.dt.int8, kind="ExternalInput")
    ws = nc.dram_tensor("ws", [nt_n, ns], mybir.dt.bfloat16, kind="ExternalInput")
    out = nc.dram_tensor("out", [TPC, OUT_F], mybir.dt.bfloat16, kind="ExternalOutput")

    n_steps = QN * nt_n
    total_steps = repeat * n_steps

    with tile.TileContext(nc) as tc:
        with (
            tc.tile_pool(name="xqpool", bufs=1) as xqpool,
            tc.tile_pool(name="xbpool", bufs=2) as xbpool,
            tc.tile_pool(name="wqpool", bufs=2) as wqpool,
            tc.tile_pool(name="wbpool", bufs=2) as wbpool,
            tc.tile_pool(name="cpool", bufs=1) as cpool,
            tc.tile_pool(name="opool", bufs=4) as opool,
            tc.tile_pool(name="pspool", bufs=4, space="PSUM") as pspool,
        ):
            ws_sb = cpool.tile([P, nt_n, ns], mybir.dt.bfloat16, name="ws_sb")
            nc.sync.dma_start(ws_sb[:], ws[:].partition_broadcast(P))
            xs_sb = cpool.tile([P, mt_n], mybir.dt.float32, name="xs_sb")
            nc.sync.dma_start(xs_sb[:], xs[:])

            def load_dequant(n):
                wq_sb = wqpool.tile([P, ko_n, ns], mybir.dt.int8, name="wq_sb")
                nc.sync.dma_start(wq_sb[:], wq[n])
                wb_sb = wbpool.tile([P, ko_n, ns], mybir.dt.bfloat16, name="wb_sb")
                nc.vector.tensor_tensor(
                    wb_sb[:],
                    wq_sb[:],
                    ws_sb[:, n, :][:, None, :].to_broadcast((P, ko_n, ns)),
                    op=mybir.AluOpType.mult,
                )
                return wb_sb

            def load_x(q):
                xq_sb = xqpool.tile([P, ko_n, QT], mybir.dt.int8, name="xq_sb")
                nc.sync.dma_start(xq_sb[:], xt[q])
                xb_sb = xbpool.tile([P, ko_n, QT], mybir.dt.bfloat16, name="xb_sb")
                nc.vector.tensor_copy(xb_sb[:], xq_sb[:])
                return xb_sb

            x_cur = load_x(0)
            wb_cur = load_dequant(0)
            for rep in range(repeat):
                for q in range(QN):
                    x_next = None
                    for n in range(nt_n):
                        s = rep * n_steps + q * nt_n + n
                        # prefetch next stripe (and next quarter's x) early so
                        # their DMAs sit ahead of this stripe's out-DMAs in
                        # the sync queue
                        wb_next = (
                            load_dequant((n + 1) % nt_n)
                            if s + 1 < total_steps
                            else None
                        )
                        if n == 0 and s + nt_n < total_steps:
                            x_next = load_x((q + 1) % QN)
                        for mo in range(mo_n):
                            ps = pspool.tile([P, ns], mybir.dt.float32)
                            for ko in range(ko_n):
                                nc.tensor.matmul(
                                    ps[:],
                                    x_cur[:, ko, mo * P : (mo + 1) * P],
                                    wb_cur[:, ko, :],
                                    start=(ko == 0),
                                    stop=(ko == ko_n - 1),
                                )
                            o_sb = opool.tile([P, ns], mybir.dt.bfloat16)
                            m = q * mo_n + mo
                            nc.scalar.activation(
                                o_sb[:],
                                ps[:],
                                mybir.ActivationFunctionType.Copy,
                                scale=xs_sb[:, m : m + 1],
                            )
                            nc.sync.dma_start(
                                out[m * P : (m + 1) * P, n * ns : (n + 1) * ns],
                                o_sb[:],
                            )
                        if wb_next is not None:
                            wb_cur = wb_next
                    if x_next is not None:
                        x_cur = x_next
    nc.finalize()
    return nc


def _host_prep(x, weight, lora_a, lora_b):
    x = np.asarray(x, dtype=np.float32)
    weight = np.asarray(weight, dtype=np.float32)
    la = np.asarray(lora_a, dtype=np.float32)
    lb = np.asarray(lora_b, dtype=np.float32)

    # Symmetric per-row absmax int8 quant-dequant, matching the reference's
    # fp32 elementwise ops bit-for-bit.
    abs_max = np.max(np.abs(weight), axis=-1, keepdims=True)
    scale = (abs_max / np.float32(127.0)).astype(np.float32)
    wqr = np.clip(
        np.round(weight / (scale + np.float32(1e-8))), -128.0, 127.0
    ).astype(np.float32)
    w_dq = wqr * scale

    w_eff = w_dq.T + np.float32(SCALING) * (la @ lb)  # [in_f, out_f]

    # Requantize W_eff to int8 with per-out-feature scales (~0.9% rel err).
    am2 = np.max(np.abs(w_eff), axis=0, keepdims=True)
    sc2 = np.maximum(
        (am2 / np.float32(127.0)).astype(np.float32), np.float32(1e-30)
    )
    wq2 = np.clip(np.round(w_eff / sc2), -128, 127).astype(np.int8)
    wq_dev = np.ascontiguousarray(
        wq2.reshape(IN_F // P, P, OUT_F // NS, NS).transpose(2, 1, 0, 3)
    )  # [8, p, ko, 512]
    ws_dev = np.ascontiguousarray(sc2.reshape(OUT_F // NS, NS)).astype(
        ml_dtypes.bfloat16
    )  # [8, 512]

    # Quantize x to int8 with per-token scales; the scales factor exactly
    # through the matmul and are re-applied on the PSUM evacuation.
    xam = np.max(np.abs(x), axis=1, keepdims=True)
    xsc = np.maximum(
        (xam / np.float32(127.0)).astype(np.float32), np.float32(1e-30)
    )
    xq = np.clip(np.round(x / xsc), -128, 127).astype(np.int8)

    xts, xss = [], []
    for c in range(N_CORES):
        sh = np.ascontiguousarray(xq[c * TPC : (c + 1) * TPC].T)  # [in_f, tpc]
        a = sh.reshape(IN_F // P, P, QN, QT).transpose(2, 1, 0, 3)  # [q, p, ko, qt]
        xts.append(np.ascontiguousarray(a))
        xss.append(
            np.ascontiguousarray(
                xsc[c * TPC : (c + 1) * TPC, 0].reshape(TPC // P, P).T
            ).astype(np.float32)
        )  # [p, m]
    return xts, xss, wq_dev, ws_dev


def kernel(x, weight, lora_a, lora_b):
    from concourse.bass_utils import run_bass_kernel_spmd

    xts, xss, wq_dev, ws_dev = _host_prep(x, weight, lora_a, lora_b)

    if "nc" not in _NC_CACHE:
        _NC_CACHE["nc"] = _build_nc()
    nc = _NC_CACHE["nc"]

    in_maps = [
        {"xt": xts[c], "xs": xss[c], "wq": wq_dev, "ws": ws_dev}
        for c in range(N_CORES)
    ]
    res = run_bass_kernel_spmd(nc, in_maps, core_ids=list(range(N_CORES)))
    out = np.concatenate(
        [res.results[c]["out"].astype(np.float32) for c in range(N_CORES)], axis=0
    )
    return out


# revision 7
# speedup vs baseline: 1.0769x; 1.0769x over previous
"""Trainium2 Bass kernel for nn_LoRALinear (quantized linear + LoRA).

reference:
    w_dq = quant_dequant_int8_per_row(weight)          # [out, in]
    out  = x @ w_dq.T + (alpha/r) * (x @ la) @ lb      # [T, out]

Math identity used here:
    out = x @ (w_dq.T + 2.0 * (la @ lb)) = x @ W_eff
        = diag(s_tok) . (x_q @ W_eff)        with x ~= diag(s_tok) . x_q

Host prep folds quant-dequant + LoRA into W_eff, re-quantizes W_eff to int8
with per-out-feature scales, and quantizes x to int8 with per-token scales
(which factor exactly through the matmul). Host<->device traffic is
minimized: each core uploads only its own 2 MiB W stripe (an on-device
AllGather into a Shared HBM buffer rebuilds the full W), x ships as int8,
and outputs return as int8 with device-computed per-(row, 512-block) absmax
scales — ~18 MiB per core round trip vs 80 MiB for a bf16-everything
kernel. Device math runs bf16 with fp32 PSUM accumulation.

Per-core device program (SPMD identical, data-parallel over tokens):
    xt  [4, 128, 32, 512] int8  - token-shard quarters (SBUF layout)
    xs  [128, 16]         f32   - per-token x scales ([p, m-tile])
    wqs [128, 32, 512]    int8  - this core's W_eff stripe (all-gathered)
    ws  [8, 512]          bf16  - per-out-feature W scales (DMA-broadcast)
    out [2048, 4096]      int8  - per-block-quantized result
    osc [16, 128, 8]      f32   - per-(row, block) absmax of the accumulator
Stripes are dequantized to bf16 on the vector engine; x quarters are
converted int8->bf16 (exact) on the vector engine; each PSUM tile is absmax-
reduced and int8-quantized on evacuation; the host applies absmax/127 and
the token scale in f32. End-to-end rel err vs the f32 reference is ~1.3e-2
(budget 2e-2).
"""

import numpy as np
import ml_dtypes

TOKENS, IN_F, OUT_F, R = 16384, 4096, 4096, 16
N_CORES = 8
TPC = TOKENS // N_CORES  # tokens per core: 2048
SCALING = 2.0  # alpha / r
P = 128
NS = 512  # out_feature stripe (one PSUM bank of f32)
QN = 4    # x quarters
QT = TPC // QN  # 512 tokens per quarter

_NC_CACHE = {}


def _build_nc(repeat=1):
    import concourse.mybir as mybir
    import concourse.tile as tile
    from concourse import bacc

    nc = bacc.Bacc("TRN2", target_bir_lowering=False, num_devices=N_CORES)
    ko_n = IN_F // P    # 32
    ns = NS
    nt_n = OUT_F // ns  # 8
    mo_n = QT // P      # 4
    mt_n = TPC // P     # 16

    xt = nc.dram_tensor("xt", [QN, P, ko_n, QT], mybir.dt.int8, kind="ExternalInput")
    xs = nc.dram_tensor("xs", [P, mt_n], mybir.dt.float32, kind="ExternalInput")
    wqs = nc.dram_tensor("wqs", [P, ko_n, ns], mybir.dt.int8, kind="ExternalInput")
    ws = nc.dram_tensor("ws", [nt_n, ns], mybir.dt.bfloat16, kind="ExternalInput")
    out = nc.dram_tensor("out", [TPC, OUT_F], mybir.dt.int8, kind="ExternalOutput")
    osc = nc.dram_tensor("osc", [mt_n, P, nt_n], mybir.dt.float32, kind="ExternalOutput")
    wqs_i = nc.dram_tensor("wqs_i", [P, ko_n, ns], mybir.dt.int8)
    wq_full = nc.dram_tensor("wq_full", [nt_n, P, ko_n, ns], mybir.dt.int8, addr_space="Shared")

    n_steps = QN * nt_n
    total_steps = repeat * n_steps

    with tile.TileContext(nc) as tc:
        with (
            tc.tile_pool(name="xqpool", bufs=1) as xqpool,
            tc.tile_pool(name="xbpool", bufs=2) as xbpool,
            tc.tile_pool(name="wqpool", bufs=2) as wqpool,
            tc.tile_pool(name="wbpool", bufs=2) as wbpool,
            tc.tile_pool(name="cpool", bufs=1) as cpool,
            tc.tile_pool(name="opool", bufs=4) as opool,
            tc.tile_pool(name="spool", bufs=8) as spool,
            tc.tile_pool(name="arpool", bufs=2) as arpool,
            tc.tile_pool(name="pspool", bufs=4, space="PSUM") as pspool,
        ):
            # bounce this core's W stripe off SBUF into an internal dram
            # tensor (collectives cannot read IO tensors), then all-gather
            # the 8 stripes so only 2 MiB of W is uploaded per core
            wbounce = cpool.tile([P, ko_n, ns], mybir.dt.int8, name="wbounce")
            nc.sync.dma_start(wbounce[:], wqs[:])
            nc.sync.dma_start(wqs_i[:], wbounce[:])
            nc.gpsimd.collective_compute(
                "AllGather",
                mybir.AluOpType.bypass,
                replica_groups=[list(range(N_CORES))],
                ins=[wqs_i[:].opt()],
                outs=[wq_full[:].opt()],
            )
            ws_sb = cpool.tile([P, nt_n, ns], mybir.dt.bfloat16, name="ws_sb")
            nc.sync.dma_start(ws_sb[:], ws[:].partition_broadcast(P))
            xs_sb = cpool.tile([P, mt_n], mybir.dt.float32, name="xs_sb")
            nc.sync.dma_start(xs_sb[:], xs[:])

            def load_dequant(n):
                wq_sb = wqpool.tile([P, ko_n, ns], mybir.dt.int8, name="wq_sb")
                nc.sync.dma_start(wq_sb[:], wq_full[n])
                wb_sb = wbpool.tile([P, ko_n, ns], mybir.dt.bfloat16, name="wb_sb")
                nc.vector.tensor_tensor(
                    wb_sb[:],
                    wq_sb[:],
                    ws_sb[:, n, :][:, None, :].to_broadcast((P, ko_n, ns)),
                    op=mybir.AluOpType.mult,
                )
                return wb_sb

            def load_x(q):
                xq_sb = xqpool.tile([P, ko_n, QT], mybir.dt.int8, name="xq_sb")
                nc.sync.dma_start(xq_sb[:], xt[q])
                xb_sb = xbpool.tile([P, ko_n, QT], mybir.dt.bfloat16, name="xb_sb")
                nc.vector.tensor_copy(xb_sb[:], xq_sb[:])
                return xb_sb

            x_cur = load_x(0)
            wb_cur = load_dequant(0)
            for rep in range(repeat):
                for q in range(QN):
                    x_next = None
                    for n in range(nt_n):
                        s = rep * n_steps + q * nt_n + n
                        # prefetch next stripe (and next quarter's x) early so
                        # their DMAs sit ahead of this stripe's out-DMAs in
                        # the sync queue
                        wb_next = (
                            load_dequant((n + 1) % nt_n)
                            if s + 1 < total_steps
                            else None
                        )
                        if n == 0 and s + nt_n < total_steps:
                            x_next = load_x((q + 1) % QN)
                        if n == 0:
                            arows = [
                                arpool.tile([P, nt_n], mybir.dt.float32, name=f"ar{i}")
                                for i in range(mo_n)
                            ]
                        for mo in range(mo_n):
                            ps = pspool.tile([P, ns], mybir.dt.float32)
                            for ko in range(ko_n):
                                nc.tensor.matmul(
                                    ps[:],
                                    x_cur[:, ko, mo * P : (mo + 1) * P],
                                    wb_cur[:, ko, :],
                                    start=(ko == 0),
                                    stop=(ko == ko_n - 1),
                                )
                            m = q * mo_n + mo
                            # int8-quantize the raw accumulator with a
                            # per-(row, 512-block) absmax scale; the host
                            # re-applies absmax/127 and the x token scale
                            nc.vector.tensor_reduce(
                                arows[mo][:, n : n + 1], ps[:],
                                mybir.AxisListType.X, mybir.AluOpType.max,
                                apply_absolute_value=True,
                            )
                            inv = spool.tile([P, 1], mybir.dt.float32, name="inv")
                            nc.vector.reciprocal(inv[:], arows[mo][:, n : n + 1])
                            nc.vector.tensor_scalar_mul(inv[:], inv[:], 127.0)
                            o_sb = opool.tile([P, ns], mybir.dt.int8)
                            nc.scalar.activation(
                                o_sb[:], ps[:],
                                mybir.ActivationFunctionType.Copy,
                                scale=inv[:],
                            )
                            nc.sync.dma_start(
                                out[m * P : (m + 1) * P, n * ns : (n + 1) * ns],
                                o_sb[:],
                            )
                            if n == nt_n - 1:
                                nc.sync.dma_start(osc[m], arows[mo][:])
                        if wb_next is not None:
                            wb_cur = wb_next
                    if x_next is not None:
                        x_cur = x_next
    nc.finalize()
    return nc


def _host_prep(x, weight, lora_a, lora_b):
    x = np.asarray(x, dtype=np.float32)
    weight = np.asarray(weight, dtype=np.float32)
    la = np.asarray(lora_a, dtype=np.float32)
    lb = np.asarray(lora_b, dtype=np.float32)

    # Symmetric per-row absmax int8 quant-dequant, matching the reference's
    # fp32 elementwise ops bit-for-bit.
    abs_max = np.max(np.abs(weight), axis=-1, keepdims=True)
    scale = (abs_max / np.float32(127.0)).astype(np.float32)
    wqr = np.clip(
        np.round(weight / (scale + np.float32(1e-8))), -128.0, 127.0
    ).astype(np.float32)
    w_dq = wqr * scale

    w_eff = w_dq.T + np.float32(SCALING) * (la @ lb)  # [in_f, out_f]

    # Requantize W_eff to int8 with per-out-feature scales (~0.9% rel err).
    am2 = np.max(np.abs(w_eff), axis=0, keepdims=True)
    sc2 = np.maximum(
        (am2 / np.float32(127.0)).astype(np.float32), np.float32(1e-30)
    )
    wq2 = np.clip(np.round(w_eff / sc2), -128, 127).astype(np.int8)
    wq_all = wq2.reshape(IN_F // P, P, OUT_F // NS, NS).transpose(2, 1, 0, 3)
    wq_shards = [np.ascontiguousarray(wq_all[c]) for c in range(N_CORES)]  # [p, ko, 512]
    ws_dev = np.ascontiguousarray(sc2.reshape(OUT_F // NS, NS)).astype(
        ml_dtypes.bfloat16
    )  # [8, 512]

    # Quantize x to int8 with per-token scales; the scales factor exactly
    # through the matmul and are re-applied on the PSUM evacuation.
    xam = np.max(np.abs(x), axis=1, keepdims=True)
    xsc = np.maximum(
        (xam / np.float32(127.0)).astype(np.float32), np.float32(1e-30)
    )
    xq = np.clip(np.round(x / xsc), -128, 127).astype(np.int8)

    xts, xss = [], []
    for c in range(N_CORES):
        sh = np.ascontiguousarray(xq[c * TPC : (c + 1) * TPC].T)  # [in_f, tpc]
        a = sh.reshape(IN_F // P, P, QN, QT).transpose(2, 1, 0, 3)  # [q, p, ko, qt]
        xts.append(np.ascontiguousarray(a))
        xss.append(
            np.ascontiguousarray(
                xsc[c * TPC : (c + 1) * TPC, 0].reshape(TPC // P, P).T
            ).astype(np.float32)
        )  # [p, m]
    return xts, xss, wq_shards, ws_dev


def kernel(x, weight, lora_a, lora_b):
    from concourse.bass_utils import run_bass_kernel_spmd

    xts, xss, wq_shards, ws_dev = _host_prep(x, weight, lora_a, lora_b)

    if "nc" not in _NC_CACHE:
        _NC_CACHE["nc"] = _build_nc()
    nc = _NC_CACHE["nc"]

    in_maps = [
        {"xt": xts[c], "xs": xss[c], "wqs": wq_shards[c], "ws": ws_dev}
        for c in range(N_CORES)
    ]
    res = run_bass_kernel_spmd(nc, in_maps, core_ids=list(range(N_CORES)))
    parts = []
    for c in range(N_CORES):
        oq = res.results[c]["out"].astype(np.float32).reshape(TPC // P, P, OUT_F // NS, NS)
        sc = res.results[c]["osc"] / np.float32(127.0)   # [16, 128, 8]
        stok = xss[c].T.reshape(TPC // P, P, 1, 1)        # [16, 128, 1, 1]
        parts.append((oq * sc[..., None] * stok).reshape(TPC, OUT_F))
    return np.concatenate(parts, axis=0).astype(np.float32)
